# revision 1
# baseline (speedup 1.0000x reference)
"""Trainium2 Bass kernel for nn_AttentionHead (B=8, S=4096, D=128).

Sharding: data-parallel over the batch dim — 1 batch element per NeuronCore,
8 cores, SPMD (same NEFF, different x slice), weights replicated. No
collectives.

The kernel is ACT-bound: softmax needs S^2 = 16.7M exps per core ~= 109us of
Activation-engine time (1.2GHz x 128 lanes). Everything else is scheduled to
hide under the exp stream and to minimize the prologue before the first exp.
Cost-model time ~190us/core vs 345us for the v1 (exp-transpose) design.

Attention (transposed scores — the key structural change vs v1):
    scoresT[jt, q_group] = kT_tile^T @ qT_group   (groups of 512 queries)
so ACT's exp writes f16 straight into the [j, q] layout PV needs for lhsT.
v1 instead DMA-xbar-transposed the exp matrix (32MB/core through the sync
ring), which made the DMA engines a co-bottleneck and serialized exp->PV.
Flat stream over n = g*16 + jp: QK pair (N=512) -> one exp [128, 1024]
(PSUM->SBUF, scale=1/sqrt(D), no max-subtraction: LN'd q/k keep scores in
f16 exp range) -> 8 PV matmuls for stream slot n-2 (lag 2 exp ops). All
four PV accumulators of a group are live at once (pvps bufs=4 x 1 bank;
qkps bufs=2 x 2 banks = all 8 banks), so PV chases exp inside its own
group and the post-stream tail is one PV slot. This exp shape is PSUM-
optimal: PV fundamentally needs 4 live banks (each i-tile's accumulation
spans its whole group) and QK/exp need double-buffering, so free-1024 exps
(128 ops x 1038ns = 133us, zero inter-exp stall in the cost model) are the
best reachable. PV's rhs is v16 with an appended ones column: column P of
each accumulator accumulates the softmax denominator for free. Normalize
by 1/rowsum on DVE; one batched output DMA per group on the sync ring
(SWDGE pays ~1us descriptor-gen per DMA on the GPSIMD engine, which would
serialize into the kernel tail; the scalar ring's DMAs issue from the ACT
sequencer and head-of-line block the exp stream).

Prologue (~51us to first exp). Hard-won scheduling facts (cost-model
traces; the tile scheduler is invariant to emission order of independent
ops, so only structure matters):
  - An xbar DmaTransposeAnt waits for ALL prior in-flight DMA completions
    (+~0.9us sem prop): the sync ring carries ONLY the x transposes plus
    the post-LN kT/qT transposes; LN params + weights go on the scalar
    ring (tiny, land first); x cast-loads (f32->f16 SWDGE) on the GPSIMD
    ring in 4 quarter tiles so the k projections chase the chunks.
  - Weight transposes run on the PE (identity matmul, PSUM bounce) — an
    xbar wt transpose ahead of the x chain costs ~3.5us of ring pipeline.
  - Phases use scoped single/dual-tag PSUM pools 8 banks deep (shallower
    pools stall the proj->evac->free slot-recycle sem chain, ~740ns/turn):
    k phase (8 banks), q phase (8), v phase (8), in that order — attention
    needs all of kT first, qT per-group-pair, v only from PV slot 2 on.
  - Per tensor: project (PE) -> stage raw f16 (ACT; also frees PSUM) ->
    bn stats from the staged f16 (DVE) -> ONE batched
    rsqrt = exp(-0.5*ln(var+eps)) per tensor (Ln/Exp share the ACT table
    set with the attention Exp -> ~1 table load total; finer-grained
    per-quarter rsqrt chains measure strictly worse) -> LN apply via DVE
    tensor_scalar (two per-partition scalars) -> xbar-transpose quarters
    -> LN weight/bias fold on GPSIMD (idle in the prologue; DVE folds
    stall its in-order queue behind each transpose's DGE+sem latency).
  - v evacuations split ACT/DVE by parity: either engine alone paces the
    in-order PE projection stream.

All SBUF pools stay open for the whole kernel (SBUF-slot reuse attaches
release waits to DMAs loading into recycled space; walrus rejects DMAs
with too many sync waits). Only PSUM pools are scoped. All xbar transposes
go on the sync HWDGE ring (concurrent transposes on the sync+scalar rings
corrupt data on HW, per v1 bisection).
"""

import math

import numpy as np

from concourse import bacc
import concourse.mybir as mybir
import concourse.tile as tile
from concourse.bass_utils import run_bass_kernel_spmd

F16 = mybir.dt.float16
F32 = mybir.dt.float32
AF = mybir.ActivationFunctionType
ALU = mybir.AluOpType

B, S, D = 8, 4096, 128
P = 128
NT = S // P    # 32 s-tiles
NQ = 4         # x/q quarter chunks
TPQ = NT // NQ  # 8 tiles per quarter
EPS = 1e-5
ISQRT_D = 1.0 / math.sqrt(D)
N_CORES = 8
GW = 512           # queries per attention group
NG = S // GW       # 8 groups
TPG = GW // P      # 4 i-tiles per group
JP = NT // 2       # 16 jt-pairs (= exp ops = stream slots) per group


def _build_attention(tc, out_d, x_d, w_d, ln_d):
    """Emit the single-core attention program.

    out_d: [S, D] f32 output AP.  x_d: [S, D] f32 input AP.
    w_d: dict q/k/v -> [D, D] f32 weight AP (torch Linear layout: out = x @ W^T).
    ln_d: dict qw/qb/kw/kb -> [D] f32 LN param APs.
    """
    nc = tc.nc

    with (
        tc.tile_pool(name="const", bufs=1) as const,
        tc.tile_pool(name="big", bufs=1) as big,
        tc.tile_pool(name="wtmp", bufs=3) as wtmp,
        tc.tile_pool(name="stat", bufs=6) as stat,
        tc.tile_pool(name="attn", bufs=2) as attn,
        tc.tile_pool(name="small", bufs=4) as small,
    ):
        # --- scalar-ring loads first: LN params + weights (land before any
        # transpose needs the DMA device)
        lnp = {}
        for key, nm in (("qw", "qnw"), ("qb", "qnb"), ("kw", "knw"),
                        ("kb", "knb")):
            t = const.tile([P, 1], F32, tag=nm, name=nm)
            nc.scalar.dma_start(t, ln_d[key][:, None])
            lnp[key] = t
        w32 = {}
        for name in ("k", "q", "v"):  # k first: head of the critical path
            w32[name] = wtmp.tile([P, P], F32, tag=f"w32_{name}",
                                  name=f"w32_{name}")
            nc.scalar.dma_start(w32[name], w_d[name])

        # --- x cast-load as ONE SWDGE DMA: chunked loads pay ~1.34us of
        # serial descriptor generation per chunk on the GPSIMD engine, and
        # the xbar transposes wait for ALL in-flight chunks anyway, so
        # chunking only delays the last byte (9.0us -> ~8.1us)
        x_r = x_d.rearrange("(t p) d -> p t d", p=P)
        x16 = big.tile([P, NT, P], F16, tag="x16", name="x16")
        nc.gpsimd.dma_start(x16, x_r)
        x16q = [x16[:, c * TPQ:(c + 1) * TPQ, :] for c in range(NQ)]

        # identity for PE transposes (ones + affine_select p==j on GPSIMD)
        ident = const.tile([P, P], F16, tag="ident", name="ident")
        nc.gpsimd.memset(ident, 1.0)
        nc.gpsimd.affine_select(ident, ident, pattern=[[-1, P]],
                                compare_op=ALU.is_equal, fill=0.0,
                                base=0, channel_multiplier=1)

        # --- weight transposes on the PE (identity matmul): keeps the sync
        # ring x-transposes-only. A wt xbar transpose ahead of the x chain
        # costs ~3.5us of ring pipeline + sem churn before the first x
        # transpose can fire.
        WT = {}
        with tc.tile_pool(name="wps", bufs=1, space="PSUM") as wps:
            for name in ("k", "q", "v"):
                w16 = wtmp.tile([P, P], F16, tag=f"w16_{name}",
                                name=f"w16_{name}")
                nc.vector.tensor_copy(w16, w32[name])
                wt_ps = wps.tile([P, P], F16, tag=f"wtp_{name}",
                                 name=f"wtp_{name}")
                nc.tensor.transpose(wt_ps, w16, ident)
                wt = const.tile([P, P], F16, tag=f"wt_{name}",
                                name=f"wt_{name}")
                nc.vector.tensor_copy(wt, wt_ps)
                WT[name] = wt

        xTq = []
        for c in range(NQ):
            xtc = big.tile([P, TPQ, P], F16, tag=f"xT_{c}", name=f"xT_{c}")
            nc.sync.dma_start_transpose(
                xtc, x16q[c].rearrange("p t d -> p (t d)"))
            xTq.append(xtc)

        # --- projections + layernorm, k-first
        rawq = big.tile([P, NT, P], F16, tag="rawq")
        rawk = big.tile([P, NT, P], F16, tag="rawk")
        v16 = big.tile([P, NT, P + 1], F16, tag="v16")  # [:, t, P] = 1.0
        nc.vector.memset(v16[:, :, P:P + 1], 1.0)
        mv = {n: big.tile([P, NT, 2], F32, tag=f"mv_{n}", name=f"mv_{n}")
              for n in ("q", "k")}
        rstd = {n: big.tile([P, NT], F32, tag=f"rstd_{n}", name=f"rstd_{n}")
                for n in ("q", "k")}
        nmr = {n: big.tile([P, NT], F32, tag=f"nmr_{n}", name=f"nmr_{n}")
               for n in ("q", "k")}
        s1 = {n: big.tile([P, NT, P], F16, tag=f"s1_{n}", name=f"s1_{n}")
              for n in ("q", "k")}
        raw = {"q": rawq, "k": rawk}

        pps_ref = [None]

        def project(name, t):
            ps = pps_ref[0].tile([P, P], F32, tag=f"p_{name}",
                                 name=f"ps_{name}")
            nc.tensor.matmul(ps, lhsT=xTq[t // TPQ][:, t % TPQ, :],
                             rhs=WT[name], start=True, stop=True)
            if name == "v":
                # split v evacuations ACT/DVE: DVE also carries the bn stats
                # (317ns/tile), ACT the staging (292); either alone would
                # pace the whole in-order PE projection stream
                # 1/3 DVE, 2/3 ACT: DVE enters the v phase ~3us late
                # (behind the q stats), and the attention pools open only
                # after the LAST evac of either engine
                if t % 3 == 0:
                    nc.vector.tensor_copy(v16[:, t, :P], ps)
                else:
                    nc.scalar.activation(v16[:, t, :P], ps, AF.Copy)
                return
            nc.scalar.activation(raw[name][:, t, :], ps, AF.Copy)
            st = stat.tile([P, 6], F32, tag="st")
            nc.vector.bn_stats(st, raw[name][:, t, :])
            nc.vector.bn_aggr(mv[name][:, t, :], st)

        def rsqrt_batch(name, lo, hi):
            # rsqrt(v) = exp(-0.5 * ln(v)), batched over tiles [lo, hi)
            vare = stat.tile([P, NT], F32, tag=f"vare_{name}{lo}",
                             name=f"vare_{name}{lo}")
            nc.vector.tensor_scalar_add(vare[:, lo:hi],
                                        mv[name][:, lo:hi, 1], EPS)
            nc.scalar.activation(rstd[name][:, lo:hi], vare[:, lo:hi], AF.Ln)
            nc.scalar.activation(rstd[name][:, lo:hi], rstd[name][:, lo:hi],
                                 AF.Exp, scale=-0.5)
            nc.vector.scalar_tensor_tensor(
                nmr[name][:, lo:hi], in0=mv[name][:, lo:hi, 0], scalar=-1.0,
                in1=rstd[name][:, lo:hi], op0=ALU.mult, op1=ALU.mult)

        def ln_apply(name, ts):
            for t in ts:
                nc.vector.tensor_scalar(
                    s1[name][:, t, :], in0=raw[name][:, t, :],
                    scalar1=rstd[name][:, t:t + 1],
                    scalar2=nmr[name][:, t:t + 1],
                    op0=ALU.mult, op1=ALU.add)

        def transpose_fold(name, src_lo_tile, n_tiles, wsb, bsb, dst_tag):
            # xbar-transpose [s, h] -> [h, s] then fold LN weight/bias on
            # GPSIMD (two per-partition scalars). Pool is idle all prologue;
            # a DVE fold would stall the in-order DVE queue ~2.5us behind
            # each transpose's DGE+DMA+sem latency, delaying later stats.
            s1f = s1[name].rearrange("p t h -> p (t h)")
            pre = big.tile([P, n_tiles, P], F16, tag=f"{dst_tag}_pre",
                           name=f"{dst_tag}_pre")
            nc.sync.dma_start_transpose(
                pre, s1f[:, src_lo_tile * P:(src_lo_tile + n_tiles) * P])
            dst = big.tile([P, n_tiles, P], F16, tag=dst_tag, name=dst_tag)
            nc.gpsimd.tensor_scalar(
                dst.rearrange("h t s -> h (t s)"),
                in0=pre.rearrange("h t s -> h (t s)"),
                scalar1=wsb, scalar2=bsb, op0=ALU.mult, op1=ALU.add)
            return dst

        kTq = [None, None, None, None]
        qTq = [None, None, None, None]

        def finish_q_quarter(c):
            # stats for tiles of quarter c are already in; rsqrt + apply +
            # transpose + fold
            rsqrt_batch("q", c * TPQ, (c + 1) * TPQ)
            ln_apply("q", range(c * TPQ, (c + 1) * TPQ))
            qTq[c] = transpose_fold("q", c * TPQ, TPQ, lnp["qw"], lnp["qb"],
                                    f"qT{c}")

        # Three scoped single-tag PSUM pools, 8 banks deep each: shallower
        # pools stall the projection pipeline on PSUM-slot recycling (the
        # proj->evac->free sem chain is ~720ns per slot turn; 8 slots keep
        # the evac engine the pacer). Phase order k -> q -> v: attention
        # needs all of kT first, qT quarter 0 next, and v only from its
        # second PV slot onward.
        with tc.tile_pool(name="ppsk", bufs=8, space="PSUM") as ppsk:
            pps_ref[0] = ppsk
            for t in range(NT):
                project("k", t)
        rsqrt_batch("k", 0, NT)
        ln_apply("k", range(NT))
        # quarter-granular: each Pool fold is ~1.5us, so the serial Pool
        # fold chain (k quarters then q quarters) finishes before the v
        # phase drains
        for qq in range(NQ):
            kTq[qq] = transpose_fold("k", qq * TPQ, TPQ, lnp["kw"],
                                     lnp["kb"], f"kT{qq}")
        with tc.tile_pool(name="ppsq", bufs=8, space="PSUM") as ppsq:
            pps_ref[0] = ppsq
            for t in range(NT):
                project("q", t)
            # rsqrt + apply in half-batches: the first half's applies
            # release qT quarter 0's transpose (the first-exp gate) a
            # little earlier; full-quarter granularity measures worse
            for hh in range(2):
                rsqrt_batch("q", hh * 16, (hh + 1) * 16)
                ln_apply("q", range(hh * 16, (hh + 1) * 16))
                for c in (2 * hh, 2 * hh + 1):
                    qTq[c] = transpose_fold("q", c * TPQ, TPQ, lnp["qw"],
                                            lnp["qb"], f"qT{c}")
        with tc.tile_pool(name="ppsv", bufs=8, space="PSUM") as ppsv:
            pps_ref[0] = ppsv
            for t in range(NT):
                project("v", t)

        # --- attention. qkps 2 x 3-bank tiles (exp ops of free 1536/1024:
        # 11 per group instead of 16 -> less per-op ACT overhead); pvps
        # 2 x 1 bank: PV runs as SEQUENTIAL full-rate chains (one i-tile's
        # 32 chunks dense, then the next) over the PREVIOUS group's resident
        # expT, so at most 2 accumulators are ever live. The last group's
        # PV runs in a short epilogue at full PE rate.
        JT_PER = 3
        TILES_PG = (NT + JT_PER - 1) // JT_PER  # 11 (10x3 + 1x2)
        with (
            tc.tile_pool(name="qkps", bufs=2, space="PSUM") as qkps,
            tc.tile_pool(name="pvps", bufs=2, space="PSUM") as pvps,
        ):
            expT = [None, None]   # per-group expT tiles (bufs=2 pool)
            osb_g = [None]

            def emit_qk_exp(g, ti):
                # QK + exp for jt tile ti (3 jt, last one 2) of group g
                if ti == 0:
                    expT[g % 2] = attn.tile([P, NT, GW], F16, tag="expt",
                                            name="expT")
                jt0 = ti * JT_PER
                w = min(JT_PER, NT - jt0)
                ps = qkps.tile([P, JT_PER, GW], F32, tag="qk", name="qk_ps")
                qg = qTq[g // 2]
                qoff = (g % 2) * TPG
                for h in range(w):
                    jt = jt0 + h
                    nc.tensor.matmul(
                        ps[:, h, :], lhsT=kTq[jt // TPQ][:, jt % TPQ, :],
                        rhs=qg[:, qoff:qoff + TPG, :].rearrange(
                            "h t s -> h (t s)"),
                        start=True, stop=True)
                nc.scalar.activation(
                    expT[g % 2][:, jt0:jt0 + w, :], ps[:, :w, :], AF.Exp,
                    scale=ISQRT_D)

            def emit_pv_chain(g, ii, c0, c1):
                # chunks [c0, c1) of i-tile ii of group g, from resident expT
                e = expT[g % 2]
                if c0 == 0:
                    pvt[ii % 2] = pvps.tile([P, P + 1], F32, tag="pv",
                                            name="pv_acc")
                ops = pvt[ii % 2]
                for c in range(c0, c1):
                    nc.tensor.matmul(
                        ops, lhsT=e[:, c, ii * P:(ii + 1) * P],
                        rhs=v16[:, c, :],
                        start=(c == 0), stop=(c == NT - 1))
                if c1 == NT:
                    if ii == 0:
                        osb_g[0] = small.tile([P, TPG, P], F32, tag="osb",
                                              name="osb")
                    rsum = small.tile([P, 1], F32, tag="rsum")
                    nc.vector.reciprocal(rsum, ops[:, P:P + 1])
                    nc.vector.tensor_scalar_mul(osb_g[0][:, ii, :],
                                                ops[:, :P], rsum)
                    if ii == 1:
                        nc.sync.dma_start(
                            out_d[g * GW:g * GW + 2 * P, :].rearrange(
                                "(t p) d -> p t d", p=P),
                            osb_g[0][:, :2, :])
                    elif ii == TPG - 1:
                        nc.sync.dma_start(
                            out_d[g * GW + 2 * P:(g + 1) * GW, :]
                            .rearrange("(t p) d -> p t d", p=P),
                            osb_g[0][:, 2:, :])

            pvt = [None, None]
            # 128 PV matmuls per group over 11 stream slots: 12 per slot,
            # walking i-tiles sequentially (ii0 chunks 0..31, then ii1, ...)
            for g in range(NG):
                pv_cursor = 0  # matmul index into group g-1's 128
                for ti in range(TILES_PG):
                    emit_qk_exp(g, ti)
                    if g == 0:
                        continue
                    end = min(128, pv_cursor + 12)
                    while pv_cursor < end:
                        ii = pv_cursor // NT
                        c0 = pv_cursor % NT
                        c1 = min(NT, c0 + (end - pv_cursor))
                        emit_pv_chain(g - 1, ii, c0, c1)
                        pv_cursor += c1 - c0
                if g > 0:
                    while pv_cursor < 128:
                        ii = pv_cursor // NT
                        emit_pv_chain(g - 1, ii, pv_cursor % NT, NT)
                        pv_cursor += NT - pv_cursor % NT
            # epilogue: last group's PV at full PE rate
            for ii in range(TPG):
                emit_pv_chain(NG - 1, ii, 0, NT)


_NC_CACHE = None


def _build():
    global _NC_CACHE
    if _NC_CACHE is not None:
        return _NC_CACHE
    nc = bacc.Bacc("TRN2", target_bir_lowering=False, debug=False)
    x = nc.dram_tensor("x", [S, D], F32, kind="ExternalInput").ap()
    wq = nc.dram_tensor("Wq", [D, D], F32, kind="ExternalInput").ap()
    wk = nc.dram_tensor("Wk", [D, D], F32, kind="ExternalInput").ap()
    wv = nc.dram_tensor("Wv", [D, D], F32, kind="ExternalInput").ap()
    qn_w = nc.dram_tensor("qn_w", [D], F32, kind="ExternalInput").ap()
    qn_b = nc.dram_tensor("qn_b", [D], F32, kind="ExternalInput").ap()
    kn_w = nc.dram_tensor("kn_w", [D], F32, kind="ExternalInput").ap()
    kn_b = nc.dram_tensor("kn_b", [D], F32, kind="ExternalInput").ap()
    out = nc.dram_tensor("out", [S, D], F32, kind="ExternalOutput").ap()
    with tile.TileContext(nc) as tc:
        _build_attention(
            tc, out, x,
            {"q": wq, "k": wk, "v": wv},
            {"qw": qn_w, "qb": qn_b, "kw": kn_w, "kb": kn_b},
        )
    nc.compile()
    _NC_CACHE = nc
    return nc


def kernel(x, Wq, Wk, Wv, qn_w, qn_b, kn_w, kn_b, _run_kwargs=None):
    nc = _build()
    x = np.asarray(x, dtype=np.float32)
    shared = {
        "Wq": np.ascontiguousarray(np.asarray(Wq, np.float32)),
        "Wk": np.ascontiguousarray(np.asarray(Wk, np.float32)),
        "Wv": np.ascontiguousarray(np.asarray(Wv, np.float32)),
        "qn_w": np.ascontiguousarray(np.asarray(qn_w, np.float32)),
        "qn_b": np.ascontiguousarray(np.asarray(qn_b, np.float32)),
        "kn_w": np.ascontiguousarray(np.asarray(kn_w, np.float32)),
        "kn_b": np.ascontiguousarray(np.asarray(kn_b, np.float32)),
    }
    in_maps = [
        {"x": np.ascontiguousarray(x[b]), **shared} for b in range(B)
    ]
    res = run_bass_kernel_spmd(nc, in_maps, core_ids=list(range(N_CORES)),
                               **(_run_kwargs or {}))
    out = np.stack([res.results[b]["out"] for b in range(B)], axis=0)
    if _run_kwargs:
        kernel.last_results = res
    return out.astype(np.float32)



# revision 6
# speedup vs baseline: 426.4120x; 426.4120x over previous
"""Trainium2 Bass kernel for nn_AttentionHead (B=8, S=4096, D=128).

Sharding: data-parallel over the batch dim — 1 batch element per NeuronCore,
8 cores, SPMD (same NEFF, different x slice), weights replicated. No
collectives.

The kernel is ACT-bound: softmax needs S^2 = 16.7M exps per core ~= 109us of
Activation-engine time (1.2GHz x 128 lanes). Everything else is scheduled to
hide under the exp stream and to minimize the prologue before the first exp.
Cost-model time ~190us/core vs 345us for the v1 (exp-transpose) design.

Attention (transposed scores — the key structural change vs v1):
    scoresT[jt, q_group] = kT_tile^T @ qT_group   (groups of 512 queries)
so ACT's exp writes f16 straight into the [j, q] layout PV needs for lhsT.
v1 instead DMA-xbar-transposed the exp matrix (32MB/core through the sync
ring), which made the DMA engines a co-bottleneck and serialized exp->PV.
Flat stream over n = g*16 + jp: QK pair (N=512) -> one exp [128, 1024]
(PSUM->SBUF, scale=1/sqrt(D), no max-subtraction: LN'd q/k keep scores in
f16 exp range) -> 8 PV matmuls for stream slot n-2 (lag 2 exp ops). All
four PV accumulators of a group are live at once (pvps bufs=4 x 1 bank;
qkps bufs=2 x 2 banks = all 8 banks), so PV chases exp inside its own
group and the post-stream tail is one PV slot. This exp shape is PSUM-
optimal: PV fundamentally needs 4 live banks (each i-tile's accumulation
spans its whole group) and QK/exp need double-buffering, so free-1024 exps
(128 ops x 1038ns = 133us, zero inter-exp stall in the cost model) are the
best reachable. PV's rhs is v16 with an appended ones column: column P of
each accumulator accumulates the softmax denominator for free. Normalize
by 1/rowsum on DVE; one batched output DMA per group on the sync ring
(SWDGE pays ~1us descriptor-gen per DMA on the GPSIMD engine, which would
serialize into the kernel tail; the scalar ring's DMAs issue from the ACT
sequencer and head-of-line block the exp stream).

Prologue (~51us to first exp). Hard-won scheduling facts (cost-model
traces; the tile scheduler is invariant to emission order of independent
ops, so only structure matters):
  - An xbar DmaTransposeAnt waits for ALL prior in-flight DMA completions
    (+~0.9us sem prop): the sync ring carries ONLY the x transposes plus
    the post-LN kT/qT transposes; LN params + weights go on the scalar
    ring (tiny, land first); x cast-loads (f32->f16 SWDGE) on the GPSIMD
    ring in 4 quarter tiles so the k projections chase the chunks.
  - Weight transposes run on the PE (identity matmul, PSUM bounce) — an
    xbar wt transpose ahead of the x chain costs ~3.5us of ring pipeline.
  - Phases use scoped single/dual-tag PSUM pools 8 banks deep (shallower
    pools stall the proj->evac->free slot-recycle sem chain, ~740ns/turn):
    k phase (8 banks), q phase (8), v phase (8), in that order — attention
    needs all of kT first, qT per-group-pair, v only from PV slot 2 on.
  - Per tensor: project (PE) -> stage raw f16 (ACT; also frees PSUM) ->
    bn stats from the staged f16 (DVE) -> ONE batched
    rsqrt = exp(-0.5*ln(var+eps)) per tensor (Ln/Exp share the ACT table
    set with the attention Exp -> ~1 table load total; finer-grained
    per-quarter rsqrt chains measure strictly worse) -> LN apply via DVE
    tensor_scalar (two per-partition scalars) -> xbar-transpose quarters
    -> LN weight/bias fold on GPSIMD (idle in the prologue; DVE folds
    stall its in-order queue behind each transpose's DGE+sem latency).
  - v evacuations split ACT/DVE by parity: either engine alone paces the
    in-order PE projection stream.

All SBUF pools stay open for the whole kernel (SBUF-slot reuse attaches
release waits to DMAs loading into recycled space; walrus rejects DMAs
with too many sync waits). Only PSUM pools are scoped. All xbar transposes
go on the sync HWDGE ring (concurrent transposes on the sync+scalar rings
corrupt data on HW, per v1 bisection).
"""

import math

import numpy as np

from concourse import bacc
import concourse.mybir as mybir
import concourse.tile as tile
from concourse.bass_utils import run_bass_kernel_spmd

F16 = mybir.dt.float16
F32 = mybir.dt.float32
AF = mybir.ActivationFunctionType
ALU = mybir.AluOpType

B, S, D = 8, 4096, 128
P = 128
NT = S // P    # 32 s-tiles
NQ = 4         # x/q quarter chunks
TPQ = NT // NQ  # 8 tiles per quarter
EPS = 1e-5
ISQRT_D = 1.0 / math.sqrt(D)
N_CORES = 8
GW = 512           # queries per attention group
NG = S // GW       # 8 groups
TPG = GW // P      # 4 i-tiles per group
JP = NT // 2       # 16 jt-pairs (= exp ops = stream slots) per group


def _build_attention(tc, out_d, x_d, w_d, ln_d):
    """Emit the single-core attention program.

    out_d: [S, D] f32 output AP.  x_d: [S, D] f32 input AP.
    w_d: dict q/k/v -> [D, D] f32 weight AP (torch Linear layout: out = x @ W^T).
    ln_d: dict qw/qb/kw/kb -> [D] f32 LN param APs.
    """
    nc = tc.nc

    with (
        tc.tile_pool(name="const", bufs=1) as const,
        tc.tile_pool(name="big", bufs=1) as big,
        tc.tile_pool(name="wtmp", bufs=3) as wtmp,
        tc.tile_pool(name="stat", bufs=6) as stat,
        tc.tile_pool(name="attn", bufs=2) as attn,
        tc.tile_pool(name="small", bufs=4) as small,
    ):
        # --- scalar-ring loads first: LN params + weights (land before any
        # transpose needs the DMA device)
        lnp = {}
        for key, nm in (("qw", "qnw"), ("qb", "qnb"), ("kw", "knw"),
                        ("kb", "knb")):
            t = const.tile([P, 1], F32, tag=nm, name=nm)
            nc.scalar.dma_start(t, ln_d[key][:, None])
            lnp[key] = t
        w32 = {}
        for name in ("k", "q", "v"):  # k first: head of the critical path
            w32[name] = wtmp.tile([P, P], F32, tag=f"w32_{name}",
                                  name=f"w32_{name}")
            nc.scalar.dma_start(w32[name], w_d[name])

        # --- x cast-load as ONE SWDGE DMA: chunked loads pay ~1.34us of
        # serial descriptor generation per chunk on the GPSIMD engine, and
        # the xbar transposes wait for ALL in-flight chunks anyway, so
        # chunking only delays the last byte (9.0us -> ~8.1us)
        x_r = x_d.rearrange("(t p) d -> p t d", p=P)
        x16 = big.tile([P, NT, P], F16, tag="x16", name="x16")
        nc.gpsimd.dma_start(x16, x_r)
        x16q = [x16[:, c * TPQ:(c + 1) * TPQ, :] for c in range(NQ)]

        # identity for PE transposes (ones + affine_select p==j on GPSIMD)
        ident = const.tile([P, P], F16, tag="ident", name="ident")
        nc.gpsimd.memset(ident, 1.0)
        nc.gpsimd.affine_select(ident, ident, pattern=[[-1, P]],
                                compare_op=ALU.is_equal, fill=0.0,
                                base=0, channel_multiplier=1)

        # --- weight transposes on the PE (identity matmul): keeps the sync
        # ring x-transposes-only. A wt xbar transpose ahead of the x chain
        # costs ~3.5us of ring pipeline + sem churn before the first x
        # transpose can fire.
        WT = {}
        with tc.tile_pool(name="wps", bufs=1, space="PSUM") as wps:
            for name in ("k", "q", "v"):
                w16 = wtmp.tile([P, P], F16, tag=f"w16_{name}",
                                name=f"w16_{name}")
                nc.vector.tensor_copy(w16, w32[name])
                wt_ps = wps.tile([P, P], F16, tag=f"wtp_{name}",
                                 name=f"wtp_{name}")
                nc.tensor.transpose(wt_ps, w16, ident)
                wt = const.tile([P, P], F16, tag=f"wt_{name}",
                                name=f"wt_{name}")
                nc.vector.tensor_copy(wt, wt_ps)
                WT[name] = wt

        xTq = []
        for c in range(NQ):
            xtc = big.tile([P, TPQ, P], F16, tag=f"xT_{c}", name=f"xT_{c}")
            nc.sync.dma_start_transpose(
                xtc, x16q[c].rearrange("p t d -> p (t d)"))
            xTq.append(xtc)

        # --- projections + layernorm, k-first
        rawq = big.tile([P, NT, P], F16, tag="rawq")
        rawk = big.tile([P, NT, P], F16, tag="rawk")
        v16 = big.tile([P, NT, P + 1], F16, tag="v16")  # [:, t, P] = 1.0
        nc.vector.memset(v16[:, :, P:P + 1], 1.0)
        mv = {n: big.tile([P, NT, 2], F32, tag=f"mv_{n}", name=f"mv_{n}")
              for n in ("q", "k")}
        rstd = {n: big.tile([P, NT], F32, tag=f"rstd_{n}", name=f"rstd_{n}")
                for n in ("q", "k")}
        nmr = {n: big.tile([P, NT], F32, tag=f"nmr_{n}", name=f"nmr_{n}")
               for n in ("q", "k")}
        s1 = {n: big.tile([P, NT, P], F16, tag=f"s1_{n}", name=f"s1_{n}")
              for n in ("q", "k")}
        raw = {"q": rawq, "k": rawk}

        pps_ref = [None]

        def project(name, t):
            ps = pps_ref[0].tile([P, P], F32, tag=f"p_{name}",
                                 name=f"ps_{name}")
            nc.tensor.matmul(ps, lhsT=xTq[t // TPQ][:, t % TPQ, :],
                             rhs=WT[name], start=True, stop=True)
            if name == "v":
                # split v evacuations ACT/DVE: DVE also carries the bn stats
                # (317ns/tile), ACT the staging (292); either alone would
                # pace the whole in-order PE projection stream
                # 1/3 DVE, 2/3 ACT: DVE enters the v phase ~3us late
                # (behind the q stats), and the attention pools open only
                # after the LAST evac of either engine
                if t % 3 == 0:
                    nc.vector.tensor_copy(v16[:, t, :P], ps)
                else:
                    nc.scalar.activation(v16[:, t, :P], ps, AF.Copy)
                return
            nc.scalar.activation(raw[name][:, t, :], ps, AF.Copy)
            st = stat.tile([P, 6], F32, tag="st")
            nc.vector.bn_stats(st, raw[name][:, t, :])
            nc.vector.bn_aggr(mv[name][:, t, :], st)

        def rsqrt_batch(name, lo, hi):
            # rsqrt(v) = exp(-0.5 * ln(v)), batched over tiles [lo, hi)
            vare = stat.tile([P, NT], F32, tag=f"vare_{name}{lo}",
                             name=f"vare_{name}{lo}")
            nc.vector.tensor_scalar_add(vare[:, lo:hi],
                                        mv[name][:, lo:hi, 1], EPS)
            nc.scalar.activation(rstd[name][:, lo:hi], vare[:, lo:hi], AF.Ln)
            nc.scalar.activation(rstd[name][:, lo:hi], rstd[name][:, lo:hi],
                                 AF.Exp, scale=-0.5)
            nc.vector.scalar_tensor_tensor(
                nmr[name][:, lo:hi], in0=mv[name][:, lo:hi, 0], scalar=-1.0,
                in1=rstd[name][:, lo:hi], op0=ALU.mult, op1=ALU.mult)

        def ln_apply(name, ts):
            for t in ts:
                nc.vector.tensor_scalar(
                    s1[name][:, t, :], in0=raw[name][:, t, :],
                    scalar1=rstd[name][:, t:t + 1],
                    scalar2=nmr[name][:, t:t + 1],
                    op0=ALU.mult, op1=ALU.add)

        def transpose_fold(name, src_lo_tile, n_tiles, wsb, bsb, dst_tag):
            # xbar-transpose [s, h] -> [h, s] then fold LN weight/bias on
            # GPSIMD (two per-partition scalars). Pool is idle all prologue;
            # a DVE fold would stall the in-order DVE queue ~2.5us behind
            # each transpose's DGE+DMA+sem latency, delaying later stats.
            s1f = s1[name].rearrange("p t h -> p (t h)")
            pre = big.tile([P, n_tiles, P], F16, tag=f"{dst_tag}_pre",
                           name=f"{dst_tag}_pre")
            nc.sync.dma_start_transpose(
                pre, s1f[:, src_lo_tile * P:(src_lo_tile + n_tiles) * P])
            dst = big.tile([P, n_tiles, P], F16, tag=dst_tag, name=dst_tag)
            nc.gpsimd.tensor_scalar(
                dst.rearrange("h t s -> h (t s)"),
                in0=pre.rearrange("h t s -> h (t s)"),
                scalar1=wsb, scalar2=bsb, op0=ALU.mult, op1=ALU.add)
            return dst

        kTq = [None, None, None, None]
        qTq = [None, None, None, None]

        def finish_q_quarter(c):
            # stats for tiles of quarter c are already in; rsqrt + apply +
            # transpose + fold
            rsqrt_batch("q", c * TPQ, (c + 1) * TPQ)
            ln_apply("q", range(c * TPQ, (c + 1) * TPQ))
            qTq[c] = transpose_fold("q", c * TPQ, TPQ, lnp["qw"], lnp["qb"],
                                    f"qT{c}")

        # Three scoped single-tag PSUM pools, 8 banks deep each: shallower
        # pools stall the projection pipeline on PSUM-slot recycling (the
        # proj->evac->free sem chain is ~720ns per slot turn; 8 slots keep
        # the evac engine the pacer). Phase order k -> q -> v: attention
        # needs all of kT first, qT quarter 0 next, and v only from its
        # second PV slot onward.
        with tc.tile_pool(name="ppsk", bufs=8, space="PSUM") as ppsk:
            pps_ref[0] = ppsk
            for t in range(NT):
                project("k", t)
        rsqrt_batch("k", 0, NT)
        ln_apply("k", range(NT))
        # quarter-granular: each Pool fold is ~1.5us, so the serial Pool
        # fold chain (k quarters then q quarters) finishes before the v
        # phase drains
        for qq in range(NQ):
            kTq[qq] = transpose_fold("k", qq * TPQ, TPQ, lnp["kw"],
                                     lnp["kb"], f"kT{qq}")
        with tc.tile_pool(name="ppsq", bufs=8, space="PSUM") as ppsq:
            pps_ref[0] = ppsq
            for t in range(NT):
                project("q", t)
            # rsqrt + apply in half-batches: the first half's applies
            # release qT quarter 0's transpose (the first-exp gate) a
            # little earlier; full-quarter granularity measures worse
            for hh in range(2):
                rsqrt_batch("q", hh * 16, (hh + 1) * 16)
                ln_apply("q", range(hh * 16, (hh + 1) * 16))
                for c in (2 * hh, 2 * hh + 1):
                    qTq[c] = transpose_fold("q", c * TPQ, TPQ, lnp["qw"],
                                            lnp["qb"], f"qT{c}")
        with tc.tile_pool(name="ppsv", bufs=8, space="PSUM") as ppsv:
            pps_ref[0] = ppsv
            for t in range(NT):
                project("v", t)

        # --- attention. qkps 2 x 3-bank tiles (exp ops of free 1536/1024:
        # 11 per group instead of 16 -> less per-op ACT overhead); pvps
        # 2 x 1 bank: PV runs as SEQUENTIAL full-rate chains (one i-tile's
        # 32 chunks dense, then the next) over the PREVIOUS group's resident
        # expT, so at most 2 accumulators are ever live. The last group's
        # PV runs in a short epilogue at full PE rate.
        JT_PER = 3
        TILES_PG = (NT + JT_PER - 1) // JT_PER  # 11 (10x3 + 1x2)
        with (
            tc.tile_pool(name="qkps", bufs=2, space="PSUM") as qkps,
            tc.tile_pool(name="pvps", bufs=2, space="PSUM") as pvps,
        ):
            expT = [None, None]   # per-group expT tiles (bufs=2 pool)
            osb_g = [None]

            def emit_qk_exp(g, ti):
                # QK + exp for jt tile ti (3 jt, last one 2) of group g
                if ti == 0:
                    expT[g % 2] = attn.tile([P, NT, GW], F16, tag="expt",
                                            name="expT")
                jt0 = ti * JT_PER
                w = min(JT_PER, NT - jt0)
                ps = qkps.tile([P, JT_PER, GW], F32, tag="qk", name="qk_ps")
                qg = qTq[g // 2]
                qoff = (g % 2) * TPG
                for h in range(w):
                    jt = jt0 + h
                    nc.tensor.matmul(
                        ps[:, h, :], lhsT=kTq[jt // TPQ][:, jt % TPQ, :],
                        rhs=qg[:, qoff:qoff + TPG, :].rearrange(
                            "h t s -> h (t s)"),
                        start=True, stop=True)
                nc.scalar.activation(
                    expT[g % 2][:, jt0:jt0 + w, :], ps[:, :w, :], AF.Exp,
                    scale=ISQRT_D)

            def emit_pv_chain(g, ii, c0, c1):
                # chunks [c0, c1) of i-tile ii of group g, from resident expT
                e = expT[g % 2]
                if c0 == 0:
                    pvt[ii % 2] = pvps.tile([P, P + 1], F32, tag="pv",
                                            name="pv_acc")
                ops = pvt[ii % 2]
                for c in range(c0, c1):
                    nc.tensor.matmul(
                        ops, lhsT=e[:, c, ii * P:(ii + 1) * P],
                        rhs=v16[:, c, :],
                        start=(c == 0), stop=(c == NT - 1))
                if c1 == NT:
                    if ii == 0:
                        osb_g[0] = small.tile([P, TPG, P], F32, tag="osb",
                                              name="osb")
                    rsum = small.tile([P, 1], F32, tag="rsum")
                    nc.vector.reciprocal(rsum, ops[:, P:P + 1])
                    nc.vector.tensor_scalar_mul(osb_g[0][:, ii, :],
                                                ops[:, :P], rsum)
                    if ii == 1:
                        nc.sync.dma_start(
                            out_d[g * GW:g * GW + 2 * P, :].rearrange(
                                "(t p) d -> p t d", p=P),
                            osb_g[0][:, :2, :])
                    elif ii == TPG - 1:
                        nc.sync.dma_start(
                            out_d[g * GW + 2 * P:(g + 1) * GW, :]
                            .rearrange("(t p) d -> p t d", p=P),
                            osb_g[0][:, 2:, :])

            pvt = [None, None]
            # 128 PV matmuls per group over 11 stream slots: 12 per slot,
            # walking i-tiles sequentially (ii0 chunks 0..31, then ii1, ...)
            for g in range(NG):
                pv_cursor = 0  # matmul index into group g-1's 128
                for ti in range(TILES_PG):
                    emit_qk_exp(g, ti)
                    if g == 0:
                        continue
                    end = min(128, pv_cursor + 12)
                    while pv_cursor < end:
                        ii = pv_cursor // NT
                        c0 = pv_cursor % NT
                        c1 = min(NT, c0 + (end - pv_cursor))
                        emit_pv_chain(g - 1, ii, c0, c1)
                        pv_cursor += c1 - c0
                if g > 0:
                    while pv_cursor < 128:
                        ii = pv_cursor // NT
                        emit_pv_chain(g - 1, ii, pv_cursor % NT, NT)
                        pv_cursor += NT - pv_cursor % NT
            # epilogue: last group's PV at full PE rate
            for ii in range(TPG):
                emit_pv_chain(NG - 1, ii, 0, NT)


_NC_CACHE = None


def _compile(nc):
    """nc.compile() with the activation-table pass pinned to the one set
    that covers every ACT function this kernel uses.

    bass's table-load pass picks, per activation, the first act_info set
    containing its function; Copy resolves to `exp_and_others` and Ln to
    `natural_log` (which lacks exp), so the k/q staging (Copy) and the
    rsqrt (Ln, Exp) phases thrash 7 table loads (~1.3us each, several on
    the prologue critical path).  `natural_log_exp_and_others` contains
    copy, identity, ln AND exp: one load total suffices.

    Set order (and hence each set's act_func_set_id, which is positional
    into act_info.json and consumed downstream by walrus) is preserved:
    we only *remove* this kernel's functions from the other sets, so the
    pass has exactly one candidate and every index keeps its meaning.
    """
    import concourse.bacc as bacc_mod

    orig = bacc_mod.get_activation_tables
    AF_ = mybir.ActivationFunctionType
    ours = {AF_.Copy, AF_.Identity, AF_.Ln, AF_.Exp}

    def filtered(module_arch):
        tables = orig(module_arch)
        pref = "natural_log_exp_and_others"
        if pref in tables and ours <= tables[pref]:
            tables = {k: (v if k == pref else v - ours)
                      for k, v in tables.items()}
        return tables

    bacc_mod.get_activation_tables = filtered
    try:
        nc.compile()
    finally:
        bacc_mod.get_activation_tables = orig


def _build():
    global _NC_CACHE
    if _NC_CACHE is not None:
        return _NC_CACHE
    nc = bacc.Bacc("TRN2", target_bir_lowering=False, debug=False)
    x = nc.dram_tensor("x", [S, D], F32, kind="ExternalInput").ap()
    wq = nc.dram_tensor("Wq", [D, D], F32, kind="ExternalInput").ap()
    wk = nc.dram_tensor("Wk", [D, D], F32, kind="ExternalInput").ap()
    wv = nc.dram_tensor("Wv", [D, D], F32, kind="ExternalInput").ap()
    qn_w = nc.dram_tensor("qn_w", [D], F32, kind="ExternalInput").ap()
    qn_b = nc.dram_tensor("qn_b", [D], F32, kind="ExternalInput").ap()
    kn_w = nc.dram_tensor("kn_w", [D], F32, kind="ExternalInput").ap()
    kn_b = nc.dram_tensor("kn_b", [D], F32, kind="ExternalInput").ap()
    out = nc.dram_tensor("out", [S, D], F32, kind="ExternalOutput").ap()
    with tile.TileContext(nc) as tc:
        _build_attention(
            tc, out, x,
            {"q": wq, "k": wk, "v": wv},
            {"qw": qn_w, "qb": qn_b, "kw": kn_w, "kb": kn_b},
        )
    _compile(nc)
    _NC_CACHE = nc
    return nc


def kernel(x, Wq, Wk, Wv, qn_w, qn_b, kn_w, kn_b, _run_kwargs=None):
    nc = _build()
    x = np.asarray(x, dtype=np.float32)
    shared = {
        "Wq": np.ascontiguousarray(np.asarray(Wq, np.float32)),
        "Wk": np.ascontiguousarray(np.asarray(Wk, np.float32)),
        "Wv": np.ascontiguousarray(np.asarray(Wv, np.float32)),
        "qn_w": np.ascontiguousarray(np.asarray(qn_w, np.float32)),
        "qn_b": np.ascontiguousarray(np.asarray(qn_b, np.float32)),
        "kn_w": np.ascontiguousarray(np.asarray(kn_w, np.float32)),
        "kn_b": np.ascontiguousarray(np.asarray(kn_b, np.float32)),
    }
    in_maps = [
        {"x": np.ascontiguousarray(x[b]), **shared} for b in range(B)
    ]
    res = run_bass_kernel_spmd(nc, in_maps, core_ids=list(range(N_CORES)),
                               **(_run_kwargs or {}))
    out = np.stack([res.results[b]["out"] for b in range(B)], axis=0)
    if _run_kwargs:
        kernel.last_results = res
    return out.astype(np.float32)

